# revision 1
# baseline (speedup 1.0000x reference)
"""IntLUTConv (1x1 conv as per-pixel GEMM) on 8 TRN2 NeuronCores.

Sharding: data-parallel over batch (B=8 -> one batch item per core), no
collectives. The kernel is HBM-bound per core (16.8 MB fp32 in + 4.2 MB
int8 out vs ~358 GB/s per-core HBM), so the design keeps the DMA queues
saturated and every compute engine under the DMA floor:

  x_b viewed as [128p, 2ct, 16384px] fp32, streamed in F-pixel chunks
  alternating between the sync HWDGE ring and the gpsimd SWDGE queue
  (two queues keep the 16 SDMA engines continuously fed)
  -> fused clip+trunc+quantize in ONE 7-op custom DVE pass
     (sign-aware pre-shift + magic-constant RNE; exact except for 31
      fp32 bit patterns out of 2^30, each off by one quant step)
  -> fp8e4 [128, 2, F] tile -> DoubleRow matmuls (K=256 per pass,
     N=256 output columns, fp32 PSUM; exact integer math)
  -> ACT Copy(scale=scale/64, bias=offset) PSUM->SBUF int8 (hardware
     cast is round-half-even + saturate, exactly matching
     clip(round(y*scale/64 + offset), -128, 127))
  -> int8 DMA out on the act HWDGE ring (tail chunks drain on the
     sync ring, which is idle once input finishes).

Steady state runs at the per-core HBM roofline (~58 us/pass measured
via reps differencing); single-shot adds ~16 us of fixed NEFF
preamble + semaphore-reset teardown.
"""
import re
import numpy as np

import concourse.bacc as bacc
import concourse.tile as tile
import concourse.mybir as mybir
from concourse.bass_utils import run_bass_kernel_spmd
from concourse.dve_spec import (
    Spec, Src0, Zero, C0, C1, C2, C3, maxx, minn, select,
    _spill_c3_to_src1,
)
from concourse.dve_ops import OPS, DveOp

B, CIN, COUT, H, W = 8, 256, 256, 128, 128
NPX = H * W            # 16384 pixels per batch item
F = 1024               # pixel chunk per pipeline stage
MAGIC = 12582912.0     # 1.5 * 2**23: float add forces RNE to integer grid
CHALF = 0.4999995231628418   # 0.5 - 2**-21: pre-shift for trunc-via-RNE
UBND = 6.500000476837158     # 7 - CHALF: upper clip bound post-shift
LBND = -7.500000476837158    # -8 + CHALF: lower clip bound post-shift

TRACE = False          # test.py sets True to collect NTFF exec time
_LAST_RESULTS = [None]


def _truncq_ref(in0, in1, s0, s1, imm2):
    return np.trunc(np.clip(in0, -8.0, 7.0))


def _register_truncq():
    """clip(-8,7) + trunc-toward-zero fused in one 7-op DVE pass.

    h   = copysign(0.5 - 2^-21, x)         (sign-aware pre-shift)
    y   = clamp(x - h, -7.5000005, 6.5000005)
    out = (y + MAGIC) - MAGIC              (RNE to integer grid == trunc)

    Shift-then-clip: any x >= 7 lands on the upper bound (rounds to 7),
    any x <= -8 on the lower (rounds to -8). Exact vs trunc(clip(x)) for
    all fp32 except 31 bit patterns of the form (integer - tiny), each
    off by one quantization step (brute-force verified over all fp32 in
    [-16,16]; ~1e-7 incidence for this input distribution).
    """
    for existing in OPS:
        if existing.name == "TRUNCQC_ANT":
            return existing
    c = Src0 < Zero
    h = select(c, Zero - C1, C1)   # +/-CHALF; Zero-C1 hoists to a Latch
    y = Src0 - h
    y1 = minn(y, C2)               # C2 = UBND (imm2)
    y2 = maxx(y1, C3)              # C3 = LBND (spilled to in1)
    body = (y2 + C0) - C0          # C0 = MAGIC (s0)
    body = _spill_c3_to_src1(body)
    op = DveOp("TRUNCQC_ANT", Spec(body=body, reference=_truncq_ref),
               subdim=False, uops_sha={})
    OPS.append(op)
    import concourse.dve_ops as dve_ops_mod
    dve_ops_mod.CUSTOM_DVE_SPECS[op.name] = op.spec
    dve_ops_mod._SUB_OPCODE_FOR_NAME[op.name] = (
        dve_ops_mod._CUSTOM_DVE_ROW_BASE + len(OPS) - 1)
    assert dve_ops_mod._SUB_OPCODE_FOR_NAME[op.name] < 0x20
    try:
        op.compile("v3")
    except ValueError as e:
        m = re.search(r'uops_sha\["v3"\]="([0-9a-f]+)"', str(e))
        if not m:
            raise
        op.uops_sha["v3"] = m.group(1)
        op.compile("v3")
    return op


def _ranges(sizes):
    out, off = [], 0
    for sz in sizes:
        out.append((off, sz))
        off += sz
    return out


def _build(scale_val: float, offset_val: float, reps: int = 1,
           F: int = F, in_chunk: int = 1024,
           work_bufs: int = 4, xq_bufs: int = 4, out_bufs: int = 4,
           ps_bufs: int = 4, out_dma: str = "scalar",
           in_dma: tuple = ("sync", "gpsimd"), use_doublerow: bool = True,
           tail_ramp: bool = False, merged_copy: bool = False,
           paired_halves: bool = False):
    op = _register_truncq()
    nc = bacc.Bacc("TRN2", target_bir_lowering=False)
    x = nc.dram_tensor("x", [CIN, NPX], mybir.dt.float32, kind="ExternalInput")
    # wt3[o][p][ki][m] = W[o*128+m, ki*128+p]
    wt3 = nc.dram_tensor("wt3", [2, 128, 2, 128], mybir.dt.float8e4,
                         kind="ExternalInput")
    out = nc.dram_tensor("out", [COUT, NPX], mybir.dt.int8, kind="ExternalOutput")

    assert in_chunk % F == 0

    with tile.TileContext(nc) as tc, \
         tc.tile_pool(name="singles", bufs=1) as singles, \
         tc.tile_pool(name="work", bufs=work_bufs) as work, \
         tc.tile_pool(name="xqp", bufs=xq_bufs) as xqp, \
         tc.tile_pool(name="outs", bufs=out_bufs) as outs, \
         tc.tile_pool(name="psum", bufs=ps_bufs, space="PSUM") as pspool:
        wt_sb = []
        for o in range(2):
            w_t = singles.tile([128, 2, 128], mybir.dt.float8e4, tag=f"wt{o}")
            nc.scalar.dma_start(out=w_t[:, :, :], in_=wt3[o, :, :, :])
            wt_sb.append(w_t)
        lb = singles.tile([128, 1], mybir.dt.float32, tag="lb")
        nc.vector.memset(lb[:, :], LBND)

        # view x [2*128, NPX] as [128, 2, NPX]: partition p, ct half, pixel
        xv = x[:, :].rearrange("(c p) n -> p c n", c=2)
        # view out [2*128, NPX] as [128, 2, NPX]: partition p, o half, pixel
        ov = out[:, :].rearrange("(o p) n -> p o n", o=2)

        # tail-ramped chunks: the last input lands sooner before the drain,
        # shortening the serial quantize->matmul->copy->store tail
        if tail_ramp == "mild":
            csizes = [in_chunk] * ((NPX - in_chunk) // in_chunk) + [512, 512]
        elif tail_ramp:
            ramp = [512, 256, 256]
            if in_chunk > 1024:
                ramp = [1024] * ((in_chunk - 1024) // 1024) + ramp
            csizes = [in_chunk] * ((NPX - in_chunk) // in_chunk) + ramp
        else:
            csizes = [in_chunk] * (NPX // in_chunk)
        assert sum(csizes) == NPX
        n_chunks = len(csizes)

        for idx, (coff, csz) in enumerate(
                [c for _ in range(reps) for c in _ranges(csizes)]):
            tail = (idx % n_chunks) >= n_chunks - 3
            # fixed tile shapes regardless of chunk size: uniform tags keep
            # the Tile semaphore set (and its serial end-of-kernel reset
            # sweep) small
            xr = work.tile([128, 2, in_chunk], mybir.dt.float32, tag="xr")
            if paired_halves:
                for ct in range(2):
                    getattr(nc, in_dma[ct % len(in_dma)]).dma_start(
                        out=xr[:, ct, :csz], in_=xv[:, ct, coff:coff + csz])
            else:
                getattr(nc, in_dma[idx % len(in_dma)]).dma_start(
                    out=xr[:, :, :csz], in_=xv[:, :, coff:coff + csz])
            for boff, bsz in _ranges([F] * (csz // F) if csz >= F else [csz]):
                xq = xqp.tile([128, 2, F], mybir.dt.float8e4, tag="xq")
                nc.vector._custom_dve(op, out=xq[:, :, :bsz],
                                      in0=xr[:, :, boff:boff + bsz],
                                      in1=lb[:, :], s0=MAGIC, s1=CHALF,
                                      imm2=UBND)
                oc = outs.tile([128, 2, F], mybir.dt.int8, tag="oc")
                if merged_copy:
                    ps2 = pspool.tile([128, 2, F], mybir.dt.float32, tag="ps")
                    for o in range(2):
                        for sub in range(bsz // 256):
                            nc.tensor.matmul(
                                ps2[:, o, sub * 256:(sub + 1) * 256],
                                wt_sb[o][:, :, :],
                                xq[:, :, sub * 256:(sub + 1) * 256],
                                start=True, stop=True,
                                perf_mode=mybir.MatmulPerfMode.DoubleRow,
                            )
                    nc.scalar.activation(
                        out=oc[:, :, :bsz], in_=ps2[:, :, :bsz],
                        func=mybir.ActivationFunctionType.Copy,
                        scale=scale_val / 64.0, bias=offset_val,
                    )
                else:
                    for o in range(2):
                        ps = pspool.tile([128, F], mybir.dt.float32, tag="ps")
                        if use_doublerow:
                            for sub in range(bsz // 256):
                                nc.tensor.matmul(
                                    ps[:, sub * 256:(sub + 1) * 256],
                                    wt_sb[o][:, :, :],
                                    xq[:, :, sub * 256:(sub + 1) * 256],
                                    start=True, stop=True,
                                    perf_mode=mybir.MatmulPerfMode.DoubleRow,
                                )
                        else:
                            for sub in range(bsz // 512):
                                for ct in range(2):
                                    nc.tensor.matmul(
                                        ps[:, sub * 512:(sub + 1) * 512],
                                        wt_sb[o][:, ct, :],
                                        xq[:, ct, sub * 512:(sub + 1) * 512],
                                        start=(ct == 0), stop=(ct == 1),
                                    )
                        nc.scalar.activation(
                            out=oc[:, o, :bsz], in_=ps[:, :bsz],
                            func=mybir.ActivationFunctionType.Copy,
                            scale=scale_val / 64.0, bias=offset_val,
                        )
                # tail outputs drain on the sync ring, idle once input is done
                out_eng = nc.sync if tail else getattr(nc, out_dma)
                out_eng.dma_start(out=ov[:, :, coff + boff:coff + boff + bsz],
                                  in_=oc[:, :, :bsz])
    nc.finalize()
    return nc


_KERNEL_CACHE: dict = {}


def _weights_host(weights: np.ndarray) -> np.ndarray:
    dt_f8 = mybir.dt.np(mybir.dt.float8e4)
    w4 = weights.reshape(2, 128, 2, 128)          # [o, m, ki, p]
    wt3 = np.ascontiguousarray(w4.transpose(0, 3, 2, 1))  # [o, p, ki, m]
    return wt3.astype(np.float32).astype(dt_f8)


def kernel(x, weights, scale, offset):
    x = np.asarray(x)
    weights = np.asarray(weights)
    sv = float(np.asarray(scale))
    ov = float(np.asarray(offset))

    key = (sv, ov)
    if key not in _KERNEL_CACHE:
        _KERNEL_CACHE[key] = _build(sv, ov)
    nc = _KERNEL_CACHE[key]

    wt_host = _weights_host(weights)
    in_maps = [
        {"x": np.ascontiguousarray(x[b].reshape(CIN, NPX)), "wt3": wt_host}
        for b in range(B)
    ]
    res = run_bass_kernel_spmd(nc, in_maps, core_ids=list(range(B)),
                               trace=TRACE)
    _LAST_RESULTS[0] = res
    return np.stack([r["out"].reshape(COUT, H, W) for r in res.results])

